# revision 7
# baseline (speedup 1.0000x reference)
"""ABMIL attention pooling on 8 TRN2 NeuronCores (Bass/Tile, SPMD).

Reference (per bag b over N=16384 instances):
    a_n   = tanh(x_n . w1) * sigmoid(x_n . w2)     gated attention score
    att   = softmax over valid n of a              (invalid -> -1e4)
    out_b = sum_n att_n * (x_n @ wf.T)             weighted pooling + proj

Folds that make this memory-bound (read xs exactly once, in fp8):
  * out = (sum_n att_n x_n) @ wf.T == sum_n att_n (x_n @ wf.T): the only
    large compute is ONE matmul y = xs @ [wf.T | 2*w1 | w2] ([N, 6]).
  * scores lie in (-1, 1), so softmax needs no max-subtraction:
    out = sum(e*y) / sum_valid(e) with e = exp(a).
  * sigmoid(x) = 0.5*(1 + tanh(x/2)); w1 is pre-scaled by 2 on the host so
    one ACT scale (0.5) serves both tanh rows.
  * invalid instances are zeroed in the packed xs, so y rows are 0 and
    e = exp(0) = 1 exactly; the host subtracts the per-bag invalid count
    from the denominator. No mask tensor reaches the device.
  * xs is staged as fp8 E3M4 (host cast): 8 MiB per core instead of 32.
    Host-sim rel_err vs f64 reference: 6.9e-3 (bf16 weights), well under
    the 2e-2 gate. E4M3 (2.3e-2) fails; E3M4's 4 mantissa bits and [2^-6,
    15.5] range fit N(0,1) data. Weights stay bf16 (tiny, exact-ish).

Matmul orientation (the key restructure vs the 104us/74us baselines):
  the 128x128 x-block is the STATIONARY operand and the [128, 6] weight
  chunk is the MOVING operand, so psY = x_blk.T @ W lands TRANSPOSED:
  [128 instances, 6] per block. Consequences:
  * LDWEIGHTS (x-block load) rides fast-weight-load (4 fp8 cols/cycle) and
    overlaps the previous matmul (ping-pong weight planes), so the PE pipe
    runs ~32+6 cycles per block-chunk instead of streaming 512 moving
    columns -- and the PE p-state ramp stops mattering.
  * all softmax/pooling post-work runs at [128, 16] shapes (instances on
    partitions): ~16 lane-cycles per op instead of 512. DVE drops from
    ~38us (shuffle-heavy [*, 512] ops) to ~3us, ACT from ~25us to ~2us.

Sharding (flash-attention style): instance dim N split 8 ways; each core
streams its shard once and emits 20 floats (per bag: sum e, sum e*y).
kernel() sums the partials and finalizes t/s on the host.
"""

import numpy as np

B, N, D, L = 4, 16384, 1024, 4
NCORES = 8
NSH = N // NCORES            # 2048 instances per bag per core
J = B * NSH                  # 8192 flattened rows per core
C8 = D // 128                # 8 contraction chunks of 128
NBLK = J // 128              # 64 n-blocks of 128 instances
BPB = NSH // 128             # 16 blocks per bag
TD = 16                      # DMA tiles (4 n-blocks each)
BPT = NBLK // TD             # 4 blocks per DMA tile
PE2 = 2                      # tiles per post chain
NCH = TD // PE2              # 8 post chains (2 per bag)

_NC_CACHE = {}


def _build_nc():
    from concourse import bacc, mybir, tile

    dt = mybir.dt
    act = mybir.ActivationFunctionType
    alu = mybir.AluOpType
    f32 = dt.float32
    bf16 = dt.bfloat16
    f8 = dt.float8e3

    nc = bacc.Bacc(
        "TRN2", target_bir_lowering=False, debug=False, num_devices=NCORES
    )

    # [tile, d-partition, (4 blocks x 8 chunks x 128 instances)] fp8 e3m4
    xsp = nc.dram_tensor("xsp", [TD, 128, BPT * C8 * 128], f8, kind="ExternalInput").ap()
    # [128, 48]: per chunk c, cols c*6..c*6+5 = [wf0..wf3, 2*w1, w2]
    wsb = nc.dram_tensor("wsb", [128, C8 * 6], bf16, kind="ExternalInput").ap()
    # per-core partials: per chain h: [h*5] = sum e; [h*5+1+l] = sum e*y_l
    outp = nc.dram_tensor("out", [1, 5 * NCH], f32, kind="ExternalOutput").ap()

    with tile.TileContext(nc) as tc:
        with (
            tc.tile_pool(name="const", bufs=1) as constp,
            tc.tile_pool(name="xs", bufs=1) as xpool,
            tc.tile_pool(name="psY", bufs=1, space="PSUM") as ypool,
            tc.tile_pool(name="sm", bufs=2) as smp,
        ):
            w_sb = constp.tile([128, C8 * 6], bf16, tag="w")
            nc.sync.dma_start(w_sb[:], wsb)
            ones = constp.tile([128, 1], f32, tag="ones")
            nc.vector.memset(ones[:], 1.0)
            # accum columns, per chain h: [h*5] denom, [h*5+1+l] numerators
            sAcc = constp.tile([128, 5 * NCH], f32, tag="sAcc")

            psY = [
                ypool.tile([128, 512], f32, tag=f"bag{b}", name=f"psY{b}")
                for b in range(B)
            ]

            # all 16 x tiles stay resident (64 KiB/partition); issue the DMAs
            # from two otherwise-idle engines so descriptors run well ahead
            # of the queues. Tile 0 is split so the PE can start on block 0
            # after ~1/4 of the first transfer.
            xts = []
            for t in range(TD):
                xt = xpool.tile(
                    [128, BPT * C8 * 128], f8, tag=f"x{t}", name=f"xt{t}"
                )
                xts.append(xt)
            nc.gpsimd.dma_start(xts[0][:, 0 : C8 * 128], xsp[0, :, 0 : C8 * 128])
            nc.gpsimd.dma_start(
                xts[0][:, C8 * 128 :], xsp[0, :, C8 * 128 :]
            )
            for t in range(1, TD):
                eng = nc.sync if t % 2 == 0 else nc.gpsimd
                eng.dma_start(xts[t][:], xsp[t])

            for t in range(TD):
                xt = xts[t]
                for bb in range(BPT):
                    blk = t * BPT + bb
                    bg, j = blk // BPB, blk % BPB
                    for c in range(C8):
                        nc.tensor.matmul(
                            psY[bg][:, j * 6 : (j + 1) * 6],
                            xt[:, (bb * C8 + c) * 128 : (bb * C8 + c + 1) * 128],
                            w_sb[:, c * 6 : (c + 1) * 6],
                            start=(c == 0),
                            stop=(c == C8 - 1),
                        )
                if t % PE2 == PE2 - 1:
                    # 8 blocks ready: post-process at [128, 8] shapes
                    h = t // PE2          # chain index; bag = h // 2
                    bg = t * BPT // BPB
                    g0 = (h % 2) * (BPB // 2)  # block offset within bag tile
                    py = psY[bg][:, g0 * 6 : (g0 + BPB // 2) * 6].rearrange(
                        "p (g k) -> p g k", k=6
                    )
                    # tanh of both score cols (s1 pre-scaled 2x on host)
                    tts = smp.tile([128, BPB // 2, 2], bf16, tag="tts")
                    nc.scalar.activation(tts[:], py[:, :, 4:6], act.Tanh, scale=0.5)
                    # v = tanh(x.w1) * (tanh(x.w2 / 2) + 1) == 2a
                    v = smp.tile([128, BPB // 2], f32, tag="v")
                    nc.vector.scalar_tensor_tensor(
                        v[:], tts[:, :, 1], 1.0, tts[:, :, 0], alu.add, alu.mult
                    )
                    # e = exp(v/2); accumulate denominator partial for chain
                    e_b = smp.tile([128, BPB // 2], bf16, tag="e")
                    nc.scalar.activation(
                        e_b[:], v[:], act.Exp, scale=0.5,
                        accum_out=sAcc[:, h * 5 : h * 5 + 1],
                    )
                    # numerators: sum_n e_n * y_nl
                    for l in range(L):
                        jnk = smp.tile([128, BPB // 2], bf16, tag=f"jnk{l}")
                        nc.vector.scalar_tensor_tensor(
                            jnk[:], py[:, :, l], 1.0, e_b[:], alu.mult, alu.mult,
                            accum_out=sAcc[:, h * 5 + 1 + l : h * 5 + 2 + l],
                        )

            # fold partitions: [1, 40] = ones.T @ sAcc, then ship out
            psOut = ypool.tile([1, 5 * NCH], f32, tag="out")
            nc.tensor.matmul(psOut[:], ones[:], sAcc[:], start=True, stop=True)
            outSb = constp.tile([1, 5 * NCH], f32, tag="outSb")
            nc.scalar.copy(outSb[:], psOut[:])
            nc.sync.dma_start(outp, outSb[:])

    nc.compile()
    return nc


def _get_nc():
    if "nc" not in _NC_CACHE:
        _NC_CACHE["nc"] = _build_nc()
    return _NC_CACHE["nc"]


def _make_in_maps(xs, valid, w1, w2, wf):
    import ml_dtypes

    validf = valid.astype(np.float32)
    xsz = xs.astype(np.float32) * validf[..., None]
    # [D, 6] = [wf.T | 2*w1 | w2], packed per 128-chunk: (p, c*6+j) = W6[c*128+p, j]
    W6 = np.concatenate(
        [wf.astype(np.float32).T, 2.0 * w1.astype(np.float32), w2.astype(np.float32)],
        axis=1,
    )
    wsb = np.ascontiguousarray(
        W6.reshape(C8, 128, 6).transpose(1, 0, 2).reshape(128, C8 * 6)
    ).astype(ml_dtypes.bfloat16)

    in_maps = []
    for c in range(NCORES):
        sh = xsz[:, c * NSH : (c + 1) * NSH, :].reshape(J, D)
        # [blk, nn, chunk, dd] -> [tile, dd, (blk-in-tile, chunk, nn)]
        a = sh.reshape(NBLK, 128, C8, 128).transpose(0, 2, 3, 1)
        a = (
            a.reshape(TD, BPT, C8, 128, 128)
            .transpose(0, 3, 1, 2, 4)
            .reshape(TD, 128, BPT * C8 * 128)
        )
        packed = np.ascontiguousarray(a).astype(ml_dtypes.float8_e3m4)
        in_maps.append({"xsp": packed, "wsb": wsb})
    return in_maps


def _run(xs, valid, w1, w2, wf, trace=False, **kwargs):
    from concourse import bass_utils

    nc = _get_nc()
    in_maps = _make_in_maps(xs, valid, w1, w2, wf)
    res = bass_utils.run_bass_kernel_spmd(
        nc, in_maps, core_ids=list(range(NCORES)), trace=trace, **kwargs
    )
    return res


def _combine(res, valid):
    """Sum per-core partial stats (flash-style unshard) and finalize t/s.

    Invalid instances were zeroed on the device input, so each contributes
    exp(0) = 1 to the denominator partials; subtract their count here.
    """
    tot = np.zeros(5 * NCH, np.float64)
    for c in range(NCORES):
        tot += np.asarray(res.results[c]["out"]).reshape(5 * NCH).astype(np.float64)
    ch = tot.reshape(B, NCH // B, 5).sum(axis=1)  # [bag, (s, t0..t3)]
    n_invalid = (~valid.astype(bool)).sum(axis=1).astype(np.float64)  # [b]
    s = ch[:, 0] - n_invalid                      # [b]
    t = ch[:, 1:]                                 # [b, l]
    return (t / s[:, None]).astype(np.float32)


def kernel(xs, valid, w1, w2, wf):
    xs, valid, w1, w2, wf = (np.asarray(a) for a in (xs, valid, w1, w2, wf))
    res = _run(xs, valid, w1, w2, wf, trace=False)
    return _combine(res, valid)


# revision 8
# speedup vs baseline: 1.0930x; 1.0930x over previous
"""ABMIL attention pooling on 8 TRN2 NeuronCores (Bass/Tile, SPMD).

Reference (per bag b over N=16384 instances):
    a_n   = tanh(x_n . w1) * sigmoid(x_n . w2)     gated attention score
    att   = softmax over valid n of a              (invalid -> -1e4)
    out_b = sum_n att_n * (x_n @ wf.T)             weighted pooling + proj

Folds that make this memory-bound (read xs exactly once, in fp8):
  * out = (sum_n att_n x_n) @ wf.T == sum_n att_n (x_n @ wf.T): the only
    large compute is ONE matmul y = xs @ [wf.T | 2*w1 | w2] ([N, 6]).
  * scores lie in (-1, 1), so softmax needs no max-subtraction:
    out = sum(e*y) / sum_valid(e) with e = exp(a).
  * sigmoid(x) = 0.5*(1 + tanh(x/2)); w1 is pre-scaled by 2 on the host so
    one ACT scale (0.5) serves both tanh rows.
  * invalid instances are zeroed in the packed xs, so y rows are 0 and
    e = exp(0) = 1 exactly; the host subtracts the per-bag invalid count
    from the denominator. No mask tensor reaches the device.
  * xs is staged as fp8 E3M4 (host cast): 8 MiB per core instead of 32.
    Host-sim rel_err vs f64 reference: 6.9e-3 (bf16 weights), well under
    the 2e-2 gate. E4M3 (2.3e-2) fails; E3M4's 4 mantissa bits and [2^-6,
    15.5] range fit N(0,1) data. Weights stay bf16 (tiny, exact-ish).

Matmul orientation (the key restructure vs the 104us/74us baselines):
  the 128x128 x-block is the STATIONARY operand and the [128, 6] weight
  chunk is the MOVING operand, so psY = x_blk.T @ W lands TRANSPOSED:
  [128 instances, 6] per block. Consequences:
  * LDWEIGHTS (x-block load) rides fast-weight-load (4 fp8 cols/cycle) and
    overlaps the previous matmul (ping-pong weight planes), so the PE pipe
    runs ~32+6 cycles per block-chunk instead of streaming 512 moving
    columns -- and the PE p-state ramp stops mattering.
  * all softmax/pooling post-work runs at [128, 16] shapes (instances on
    partitions): ~16 lane-cycles per op instead of 512. DVE drops from
    ~38us (shuffle-heavy [*, 512] ops) to ~3us, ACT from ~25us to ~2us.

Sharding (flash-attention style): instance dim N split 8 ways; each core
streams its shard once and emits 20 floats (per bag: sum e, sum e*y).
kernel() sums the partials and finalizes t/s on the host.
"""

import numpy as np

B, N, D, L = 4, 16384, 1024, 4
NCORES = 8
NSH = N // NCORES            # 2048 instances per bag per core
J = B * NSH                  # 8192 flattened rows per core
C8 = D // 128                # 8 contraction chunks of 128
NBLK = J // 128              # 64 n-blocks of 128 instances
BPB = NSH // 128             # 16 blocks per bag
TD = 16                      # DMA tiles (4 n-blocks each)
BPT = NBLK // TD             # 4 blocks per DMA tile
PE2 = 2                      # tiles per post chain
NCH = TD // PE2              # 8 post chains (2 per bag)

_NC_CACHE = {}


def _build_nc():
    from concourse import bacc, mybir, tile

    dt = mybir.dt
    act = mybir.ActivationFunctionType
    alu = mybir.AluOpType
    f32 = dt.float32
    bf16 = dt.bfloat16
    f8 = dt.float8e3

    nc = bacc.Bacc(
        "TRN2", target_bir_lowering=False, debug=False, num_devices=NCORES
    )

    # [tile, d-partition, (4 blocks x 8 chunks x 128 instances)] fp8 e3m4
    xsp = nc.dram_tensor("xsp", [TD, 128, BPT * C8 * 128], f8, kind="ExternalInput").ap()
    # [128, 48]: per chunk c, cols c*6..c*6+5 = [wf0..wf3, 2*w1, w2]
    wsb = nc.dram_tensor("wsb", [128, C8 * 6], bf16, kind="ExternalInput").ap()
    # per-core partials: per chain h: [h*5] = sum e; [h*5+1+l] = sum e*y_l
    outp = nc.dram_tensor("out", [1, 5 * NCH], f32, kind="ExternalOutput").ap()

    with tile.TileContext(nc) as tc:
        with (
            tc.tile_pool(name="const", bufs=1) as constp,
            tc.tile_pool(name="xs", bufs=1) as xpool,
            tc.tile_pool(name="psY", bufs=1, space="PSUM") as ypool,
            tc.tile_pool(name="sm", bufs=2) as smp,
        ):
            w_sb = constp.tile([128, C8 * 6], bf16, tag="w")
            nc.sync.dma_start(w_sb[:], wsb)
            ones = constp.tile([128, 1], f32, tag="ones")
            nc.vector.memset(ones[:], 1.0)
            # accum columns, per chain h: [h*5] denom, [h*5+1+l] numerators
            sAcc = constp.tile([128, 5 * NCH], f32, tag="sAcc")

            psY = [
                ypool.tile([128, 512], f32, tag=f"bag{b}", name=f"psY{b}")
                for b in range(B)
            ]

            # all 16 x tiles stay resident (64 KiB/partition); issue the DMAs
            # from two otherwise-idle engines so descriptors run well ahead
            # of the queues. Tile 0 is split so the PE can start on block 0
            # after ~1/4 of the first transfer.
            xts = []
            for t in range(TD):
                xt = xpool.tile(
                    [128, BPT * C8 * 128], f8, tag=f"x{t}", name=f"xt{t}"
                )
                xts.append(xt)
            nc.gpsimd.dma_start(xts[0][:, 0 : C8 * 128], xsp[0, :, 0 : C8 * 128])
            nc.gpsimd.dma_start(
                xts[0][:, C8 * 128 :], xsp[0, :, C8 * 128 :]
            )
            for t in range(1, TD):
                nc.gpsimd.dma_start(xts[t][:], xsp[t])

            for t in range(TD):
                xt = xts[t]
                for bb in range(BPT):
                    blk = t * BPT + bb
                    bg, j = blk // BPB, blk % BPB
                    for c in range(C8):
                        nc.tensor.matmul(
                            psY[bg][:, j * 6 : (j + 1) * 6],
                            xt[:, (bb * C8 + c) * 128 : (bb * C8 + c + 1) * 128],
                            w_sb[:, c * 6 : (c + 1) * 6],
                            start=(c == 0),
                            stop=(c == C8 - 1),
                        )
                if t % PE2 == PE2 - 1:
                    # 8 blocks ready: post-process at [128, 8] shapes
                    h = t // PE2          # chain index; bag = h // 2
                    bg = t * BPT // BPB
                    g0 = (h % 2) * (BPB // 2)  # block offset within bag tile
                    py = psY[bg][:, g0 * 6 : (g0 + BPB // 2) * 6].rearrange(
                        "p (g k) -> p g k", k=6
                    )
                    # tanh of both score cols (s1 pre-scaled 2x on host)
                    tts = smp.tile([128, BPB // 2, 2], bf16, tag="tts")
                    nc.scalar.activation(tts[:], py[:, :, 4:6], act.Tanh, scale=0.5)
                    # v = tanh(x.w1) * (tanh(x.w2 / 2) + 1) == 2a
                    v = smp.tile([128, BPB // 2], f32, tag="v")
                    nc.vector.scalar_tensor_tensor(
                        v[:], tts[:, :, 1], 1.0, tts[:, :, 0], alu.add, alu.mult
                    )
                    # e = exp(v/2); accumulate denominator partial for chain
                    e_b = smp.tile([128, BPB // 2], bf16, tag="e")
                    nc.scalar.activation(
                        e_b[:], v[:], act.Exp, scale=0.5,
                        accum_out=sAcc[:, h * 5 : h * 5 + 1],
                    )
                    # numerators: sum_n e_n * y_nl
                    for l in range(L):
                        jnk = smp.tile([128, BPB // 2], bf16, tag=f"jnk{l}")
                        nc.vector.scalar_tensor_tensor(
                            jnk[:], py[:, :, l], 1.0, e_b[:], alu.mult, alu.mult,
                            accum_out=sAcc[:, h * 5 + 1 + l : h * 5 + 2 + l],
                        )

            # fold partitions: [1, 40] = ones.T @ sAcc, then ship out
            psOut = ypool.tile([1, 5 * NCH], f32, tag="out")
            nc.tensor.matmul(psOut[:], ones[:], sAcc[:], start=True, stop=True)
            outSb = constp.tile([1, 5 * NCH], f32, tag="outSb")
            nc.scalar.copy(outSb[:], psOut[:])
            nc.sync.dma_start(outp, outSb[:])

    nc.compile()
    return nc


def _get_nc():
    if "nc" not in _NC_CACHE:
        _NC_CACHE["nc"] = _build_nc()
    return _NC_CACHE["nc"]


def _make_in_maps(xs, valid, w1, w2, wf):
    import ml_dtypes

    validf = valid.astype(np.float32)
    xsz = xs.astype(np.float32) * validf[..., None]
    # [D, 6] = [wf.T | 2*w1 | w2], packed per 128-chunk: (p, c*6+j) = W6[c*128+p, j]
    W6 = np.concatenate(
        [wf.astype(np.float32).T, 2.0 * w1.astype(np.float32), w2.astype(np.float32)],
        axis=1,
    )
    wsb = np.ascontiguousarray(
        W6.reshape(C8, 128, 6).transpose(1, 0, 2).reshape(128, C8 * 6)
    ).astype(ml_dtypes.bfloat16)

    in_maps = []
    for c in range(NCORES):
        sh = xsz[:, c * NSH : (c + 1) * NSH, :].reshape(J, D)
        # [blk, nn, chunk, dd] -> [tile, dd, (blk-in-tile, chunk, nn)]
        a = sh.reshape(NBLK, 128, C8, 128).transpose(0, 2, 3, 1)
        a = (
            a.reshape(TD, BPT, C8, 128, 128)
            .transpose(0, 3, 1, 2, 4)
            .reshape(TD, 128, BPT * C8 * 128)
        )
        packed = np.ascontiguousarray(a).astype(ml_dtypes.float8_e3m4)
        in_maps.append({"xsp": packed, "wsb": wsb})
    return in_maps


def _run(xs, valid, w1, w2, wf, trace=False, **kwargs):
    from concourse import bass_utils

    nc = _get_nc()
    in_maps = _make_in_maps(xs, valid, w1, w2, wf)
    res = bass_utils.run_bass_kernel_spmd(
        nc, in_maps, core_ids=list(range(NCORES)), trace=trace, **kwargs
    )
    return res


def _combine(res, valid):
    """Sum per-core partial stats (flash-style unshard) and finalize t/s.

    Invalid instances were zeroed on the device input, so each contributes
    exp(0) = 1 to the denominator partials; subtract their count here.
    """
    tot = np.zeros(5 * NCH, np.float64)
    for c in range(NCORES):
        tot += np.asarray(res.results[c]["out"]).reshape(5 * NCH).astype(np.float64)
    ch = tot.reshape(B, NCH // B, 5).sum(axis=1)  # [bag, (s, t0..t3)]
    n_invalid = (~valid.astype(bool)).sum(axis=1).astype(np.float64)  # [b]
    s = ch[:, 0] - n_invalid                      # [b]
    t = ch[:, 1:]                                 # [b, l]
    return (t / s[:, None]).astype(np.float32)


def kernel(xs, valid, w1, w2, wf):
    xs, valid, w1, w2, wf = (np.asarray(a) for a in (xs, valid, w1, w2, wf))
    res = _run(xs, valid, w1, w2, wf, trace=False)
    return _combine(res, valid)


# revision 11
# speedup vs baseline: 1.1326x; 1.0362x over previous
"""ABMIL attention pooling on 8 TRN2 NeuronCores (Bass/Tile, SPMD).

Reference (per bag b over N=16384 instances):
    a_n   = tanh(x_n . w1) * sigmoid(x_n . w2)     gated attention score
    att   = softmax over valid n of a              (invalid -> -1e4)
    out_b = sum_n att_n * (x_n @ wf.T)             weighted pooling + proj

Folds that make this memory-bound (read xs exactly once, in fp8):
  * out = (sum_n att_n x_n) @ wf.T == sum_n att_n (x_n @ wf.T): the only
    large compute is ONE matmul y = xs @ [wf.T | 2*w1 | w2] ([N, 6]).
  * scores lie in (-1, 1), so softmax needs no max-subtraction:
    out = sum(e*y) / sum_valid(e) with e = exp(a).
  * sigmoid(x) = 0.5*(1 + tanh(x/2)); w1 is pre-scaled by 2 on the host so
    one ACT scale (0.5) serves both tanh rows.
  * invalid instances are zeroed in the packed xs, so y rows are 0 and
    e = exp(0) = 1 exactly; the host subtracts the per-bag invalid count
    from the denominator. No mask tensor reaches the device.
  * xs is staged as fp8 E3M4 (host cast): 8 MiB per core instead of 32.
    Host-sim rel_err vs f64 reference: 6.9e-3 (bf16 weights), well under
    the 2e-2 gate. E4M3 (2.3e-2) fails; E3M4's 4 mantissa bits and [2^-6,
    15.5] range fit N(0,1) data. Weights stay bf16 (tiny, exact-ish).

Matmul orientation (the key restructure vs the 104us/74us baselines):
  the 128x128 x-block is the STATIONARY operand and the [128, 6] weight
  chunk is the MOVING operand, so psY = x_blk.T @ W lands TRANSPOSED:
  [128 instances, 6] per block. Consequences:
  * LDWEIGHTS (x-block load) rides fast-weight-load (4 fp8 cols/cycle) and
    overlaps the previous matmul (ping-pong weight planes), so the PE pipe
    runs ~32+6 cycles per block-chunk instead of streaming 512 moving
    columns -- and the PE p-state ramp stops mattering.
  * all softmax/pooling post-work runs at [128, 16] shapes (instances on
    partitions): ~16 lane-cycles per op instead of 512. DVE drops from
    ~38us (shuffle-heavy [*, 512] ops) to ~3us, ACT from ~25us to ~2us.

Sharding (flash-attention style): instance dim N split 8 ways; each core
streams its shard once and emits 20 floats (per bag: sum e, sum e*y).
kernel() sums the partials and finalizes t/s on the host.
"""

import numpy as np

B, N, D, L = 4, 16384, 1024, 4
NCORES = 8
NSH = N // NCORES            # 2048 instances per bag per core
J = B * NSH                  # 8192 flattened rows per core
C8 = D // 128                # 8 contraction chunks of 128
NBLK = J // 128              # 64 n-blocks of 128 instances
BPB = NSH // 128             # 16 blocks per bag
TD = 16                      # DMA tiles (4 n-blocks each)
BPT = NBLK // TD             # 4 blocks per DMA tile
PE2 = 2                      # tiles per post chain
NCH = TD // PE2              # 8 post chains (2 per bag)

_NC_CACHE = {}


def _build_nc():
    from concourse import bacc, mybir, tile

    dt = mybir.dt
    act = mybir.ActivationFunctionType
    alu = mybir.AluOpType
    f32 = dt.float32
    bf16 = dt.bfloat16
    f8 = dt.float8e3

    nc = bacc.Bacc(
        "TRN2", target_bir_lowering=False, debug=False, num_devices=NCORES
    )

    # [tile, d-partition, (4 blocks x 8 chunks x 128 instances)] fp8 e3m4
    xsp = nc.dram_tensor("xsp", [TD, 128, BPT * C8 * 128], f8, kind="ExternalInput").ap()
    # [128, 48]: per chunk c, cols c*6..c*6+5 = [wf0..wf3, 2*w1, w2]
    wsb = nc.dram_tensor("wsb", [128, C8 * 6], bf16, kind="ExternalInput").ap()
    # per-core partials: per chain h: [h*5] = sum e; [h*5+1+l] = sum e*y_l
    outp = nc.dram_tensor("out", [1, 5 * NCH], f32, kind="ExternalOutput").ap()

    with tile.TileContext(nc) as tc:
        with (
            tc.tile_pool(name="const", bufs=1) as constp,
            tc.tile_pool(name="xs", bufs=1) as xpool,
            tc.tile_pool(name="psY", bufs=1, space="PSUM") as ypool,
            tc.tile_pool(name="sm", bufs=2) as smp,
        ):
            w_sb = constp.tile([128, C8 * 6], bf16, tag="w")
            nc.sync.dma_start(w_sb[:], wsb)
            ones = constp.tile([128, 1], f32, tag="ones")
            nc.vector.memset(ones[:], 1.0)
            # accum columns, per chain h: [h*5] denom, [h*5+1+l] numerators
            sAcc = constp.tile([128, 5 * NCH], f32, tag="sAcc")

            # one PSUM bank per post chain (8 blocks x 6 cols): a chain's
            # reads never share a tile with the next tiles' matmul writes,
            # so the PE is never WAR-stalled against post-processing
            psY = [
                ypool.tile([128, 64], f32, tag=f"ch{h}", name=f"psY{h}")
                for h in range(NCH)
            ]

            # all 16 x tiles stay resident (64 KiB/partition); issue the DMAs
            # from two otherwise-idle engines so descriptors run well ahead
            # of the queues. Tile 0 is split so the PE can start on block 0
            # after ~1/4 of the first transfer.
            xts = []
            for t in range(TD):
                xt = xpool.tile(
                    [128, BPT * C8 * 128], f8, tag=f"x{t}", name=f"xt{t}"
                )
                xts.append(xt)
            nc.gpsimd.dma_start(xts[0][:, 0 : C8 * 128], xsp[0, :, 0 : C8 * 128])
            nc.gpsimd.dma_start(
                xts[0][:, C8 * 128 :], xsp[0, :, C8 * 128 :]
            )
            for t in range(1, TD):
                nc.gpsimd.dma_start(xts[t][:], xsp[t])

            for t in range(TD):
                xt = xts[t]
                for bb in range(BPT):
                    blk = t * BPT + bb
                    ch, j = blk // (BPB // 2), blk % (BPB // 2)
                    for c in range(C8):
                        nc.tensor.matmul(
                            psY[ch][:, j * 6 : (j + 1) * 6],
                            xt[:, (bb * C8 + c) * 128 : (bb * C8 + c + 1) * 128],
                            w_sb[:, c * 6 : (c + 1) * 6],
                            start=(c == 0),
                            stop=(c == C8 - 1),
                        )
                if t % PE2 == PE2 - 1:
                    # 8 blocks ready: post-process at [128, 8] shapes
                    h = t // PE2          # chain index; bag = h // 2
                    py = psY[h][:, 0 : (BPB // 2) * 6].rearrange(
                        "p (g k) -> p g k", k=6
                    )
                    # tanh of both score cols (s1 pre-scaled 2x on host)
                    tts = smp.tile([128, BPB // 2, 2], bf16, tag="tts")
                    nc.scalar.activation(tts[:], py[:, :, 4:6], act.Tanh, scale=0.5)
                    # v = tanh(x.w1) * (tanh(x.w2 / 2) + 1) == 2a
                    v = smp.tile([128, BPB // 2], f32, tag="v")
                    nc.vector.scalar_tensor_tensor(
                        v[:], tts[:, :, 1], 1.0, tts[:, :, 0], alu.add, alu.mult
                    )
                    # e = exp(v/2); accumulate denominator partial for chain
                    e_b = smp.tile([128, BPB // 2], bf16, tag="e")
                    nc.scalar.activation(
                        e_b[:], v[:], act.Exp, scale=0.5,
                        accum_out=sAcc[:, h * 5 : h * 5 + 1],
                    )
                    # numerators: sum_n e_n * y_nl
                    for l in range(L):
                        jnk = smp.tile([128, BPB // 2], bf16, tag=f"jnk{l}")
                        nc.vector.scalar_tensor_tensor(
                            jnk[:], py[:, :, l], 1.0, e_b[:], alu.mult, alu.mult,
                            accum_out=sAcc[:, h * 5 + 1 + l : h * 5 + 2 + l],
                        )

            # fold partitions: [1, 40] = ones.T @ sAcc, then ship out
            # (reuses chain 0's PSUM bank -- all 8 banks hold chain tiles)
            psOut = ypool.tile([1, 5 * NCH], f32, tag="ch0")
            nc.tensor.matmul(psOut[:], ones[:], sAcc[:], start=True, stop=True)
            outSb = constp.tile([1, 5 * NCH], f32, tag="outSb")
            nc.scalar.copy(outSb[:], psOut[:])
            nc.sync.dma_start(outp, outSb[:])

    nc.compile()
    return nc


def _get_nc():
    if "nc" not in _NC_CACHE:
        _NC_CACHE["nc"] = _build_nc()
    return _NC_CACHE["nc"]


def _make_in_maps(xs, valid, w1, w2, wf):
    import ml_dtypes

    validf = valid.astype(np.float32)
    xsz = xs.astype(np.float32) * validf[..., None]
    # [D, 6] = [wf.T | 2*w1 | w2], packed per 128-chunk: (p, c*6+j) = W6[c*128+p, j]
    W6 = np.concatenate(
        [wf.astype(np.float32).T, 2.0 * w1.astype(np.float32), w2.astype(np.float32)],
        axis=1,
    )
    wsb = np.ascontiguousarray(
        W6.reshape(C8, 128, 6).transpose(1, 0, 2).reshape(128, C8 * 6)
    ).astype(ml_dtypes.bfloat16)

    in_maps = []
    for c in range(NCORES):
        sh = xsz[:, c * NSH : (c + 1) * NSH, :].reshape(J, D)
        # [blk, nn, chunk, dd] -> [tile, dd, (blk-in-tile, chunk, nn)]
        a = sh.reshape(NBLK, 128, C8, 128).transpose(0, 2, 3, 1)
        a = (
            a.reshape(TD, BPT, C8, 128, 128)
            .transpose(0, 3, 1, 2, 4)
            .reshape(TD, 128, BPT * C8 * 128)
        )
        packed = np.ascontiguousarray(a).astype(ml_dtypes.float8_e3m4)
        in_maps.append({"xsp": packed, "wsb": wsb})
    return in_maps


def _run(xs, valid, w1, w2, wf, trace=False, **kwargs):
    from concourse import bass_utils

    nc = _get_nc()
    in_maps = _make_in_maps(xs, valid, w1, w2, wf)
    res = bass_utils.run_bass_kernel_spmd(
        nc, in_maps, core_ids=list(range(NCORES)), trace=trace, **kwargs
    )
    return res


def _combine(res, valid):
    """Sum per-core partial stats (flash-style unshard) and finalize t/s.

    Invalid instances were zeroed on the device input, so each contributes
    exp(0) = 1 to the denominator partials; subtract their count here.
    """
    tot = np.zeros(5 * NCH, np.float64)
    for c in range(NCORES):
        tot += np.asarray(res.results[c]["out"]).reshape(5 * NCH).astype(np.float64)
    ch = tot.reshape(B, NCH // B, 5).sum(axis=1)  # [bag, (s, t0..t3)]
    n_invalid = (~valid.astype(bool)).sum(axis=1).astype(np.float64)  # [b]
    s = ch[:, 0] - n_invalid                      # [b]
    t = ch[:, 1:]                                 # [b, l]
    return (t / s[:, None]).astype(np.float32)


def kernel(xs, valid, w1, w2, wf):
    xs, valid, w1, w2, wf = (np.asarray(a) for a in (xs, valid, w1, w2, wf))
    res = _run(xs, valid, w1, w2, wf, trace=False)
    return _combine(res, valid)


# revision 12
# speedup vs baseline: 1.1754x; 1.0378x over previous
"""ABMIL attention pooling on 8 TRN2 NeuronCores (Bass/Tile, SPMD).

Reference (per bag b over N=16384 instances):
    a_n   = tanh(x_n . w1) * sigmoid(x_n . w2)     gated attention score
    att   = softmax over valid n of a              (invalid -> -1e4)
    out_b = sum_n att_n * (x_n @ wf.T)             weighted pooling + proj

Folds that make this memory-bound (read xs exactly once, in fp8):
  * out = (sum_n att_n x_n) @ wf.T == sum_n att_n (x_n @ wf.T): the only
    large compute is ONE matmul y = xs @ [wf.T | 2*w1 | w2] ([N, 6]).
  * scores lie in (-1, 1), so softmax needs no max-subtraction:
    out = sum(e*y) / sum_valid(e) with e = exp(a).
  * sigmoid(x) = 0.5*(1 + tanh(x/2)); w1 is pre-scaled by 2 on the host so
    one ACT scale (0.5) serves both tanh rows.
  * invalid instances are zeroed in the packed xs, so y rows are 0 and
    e = exp(0) = 1 exactly; the host subtracts the per-bag invalid count
    from the denominator. No mask tensor reaches the device.
  * xs is staged as fp8 E3M4 (host cast): 8 MiB per core instead of 32.
    Host-sim rel_err vs f64 reference: 6.9e-3 (bf16 weights), well under
    the 2e-2 gate. E4M3 (2.3e-2) fails; E3M4's 4 mantissa bits and [2^-6,
    15.5] range fit N(0,1) data. Weights stay bf16 (tiny, exact-ish).

Matmul orientation (the key restructure vs the 104us/74us baselines):
  the 128x128 x-block is the STATIONARY operand and the [128, 6] weight
  chunk is the MOVING operand, so psY = x_blk.T @ W lands TRANSPOSED:
  [128 instances, 6] per block. Consequences:
  * LDWEIGHTS (x-block load) rides fast-weight-load (4 fp8 cols/cycle) and
    overlaps the previous matmul (ping-pong weight planes), so the PE pipe
    runs ~32+6 cycles per block-chunk instead of streaming 512 moving
    columns -- and the PE p-state ramp stops mattering.
  * all softmax/pooling post-work runs at [128, 16] shapes (instances on
    partitions): ~16 lane-cycles per op instead of 512. DVE drops from
    ~38us (shuffle-heavy [*, 512] ops) to ~3us, ACT from ~25us to ~2us.

Sharding (flash-attention style): instance dim N split 8 ways; each core
streams its shard once and emits 20 floats (per bag: sum e, sum e*y).
kernel() sums the partials and finalizes t/s on the host.
"""

import numpy as np

B, N, D, L = 4, 16384, 1024, 4
NCORES = 8
NSH = N // NCORES            # 2048 instances per bag per core
J = B * NSH                  # 8192 flattened rows per core
C8 = D // 128                # 8 contraction chunks of 128
NBLK = J // 128              # 64 n-blocks of 128 instances
BPB = NSH // 128             # 16 blocks per bag
TD = 16                      # DMA tiles (4 n-blocks each)
BPT = NBLK // TD             # 4 blocks per DMA tile
PE2 = 2                      # tiles per post chain
NCH = TD // PE2              # 8 post chains (2 per bag)

_NC_CACHE = {}


def _build_nc():
    from concourse import bacc, mybir, tile

    dt = mybir.dt
    act = mybir.ActivationFunctionType
    alu = mybir.AluOpType
    f32 = dt.float32
    bf16 = dt.bfloat16
    f8 = dt.float8e3

    nc = bacc.Bacc(
        "TRN2", target_bir_lowering=False, debug=False, num_devices=NCORES
    )

    # [tile, d-partition, (4 blocks x 8 chunks x 128 instances)] fp8 e3m4
    xsp = nc.dram_tensor("xsp", [TD, 128, BPT * C8 * 128], f8, kind="ExternalInput").ap()
    # [128, 48]: per chunk c, cols c*6..c*6+5 = [wf0..wf3, 2*w1, w2]
    wsb = nc.dram_tensor("wsb", [128, C8 * 6], bf16, kind="ExternalInput").ap()
    # per-core partials: per chain h: [h*5] = sum e; [h*5+1+l] = sum e*y_l
    outp = nc.dram_tensor("out", [1, 5 * NCH], f32, kind="ExternalOutput").ap()

    with tile.TileContext(nc) as tc:
        with (
            tc.tile_pool(name="const", bufs=1) as constp,
            tc.tile_pool(name="xs", bufs=1) as xpool,
            tc.tile_pool(name="psY", bufs=1, space="PSUM") as ypool,
            tc.tile_pool(name="sm", bufs=2) as smp,
        ):
            w_sb = constp.tile([128, C8 * 6], bf16, tag="w")
            nc.sync.dma_start(w_sb[:], wsb)
            ones = constp.tile([128, 1], f32, tag="ones")
            nc.vector.memset(ones[:], 1.0)
            # accum columns, per chain h: [h*5] denom, [h*5+1+l] numerators
            sAcc = constp.tile([128, 5 * NCH], f32, tag="sAcc")

            # one PSUM bank per post chain (8 blocks x 6 cols): a chain's
            # reads never share a tile with the next tiles' matmul writes,
            # so the PE is never WAR-stalled against post-processing
            psY = [
                ypool.tile([128, 64], f32, tag=f"ch{h}", name=f"psY{h}")
                for h in range(NCH)
            ]

            # all 16 x tiles stay resident (64 KiB/partition) and their DMAs
            # are all issued upfront from gpsimd, so no mid-stream issue
            # gating: the queues drain the full 8 MiB back to back.
            xts = []
            for t in range(TD):
                xt = xpool.tile(
                    [128, BPT * C8 * 128], f8, tag=f"x{t}", name=f"xt{t}"
                )
                xts.append(xt)
                nc.gpsimd.dma_start(xt[:], xsp[t])

            for t in range(TD):
                xt = xts[t]
                for bb in range(BPT):
                    blk = t * BPT + bb
                    ch, j = blk // (BPB // 2), blk % (BPB // 2)
                    for c in range(C8):
                        nc.tensor.matmul(
                            psY[ch][:, j * 6 : (j + 1) * 6],
                            xt[:, (bb * C8 + c) * 128 : (bb * C8 + c + 1) * 128],
                            w_sb[:, c * 6 : (c + 1) * 6],
                            start=(c == 0),
                            stop=(c == C8 - 1),
                        )
                if t % PE2 == PE2 - 1:
                    # 8 blocks ready: post-process at [128, 8] shapes
                    h = t // PE2          # chain index; bag = h // 2
                    py = psY[h][:, 0 : (BPB // 2) * 6].rearrange(
                        "p (g k) -> p g k", k=6
                    )
                    # tanh of both score cols (s1 pre-scaled 2x on host)
                    tts = smp.tile([128, BPB // 2, 2], bf16, tag="tts")
                    nc.scalar.activation(tts[:], py[:, :, 4:6], act.Tanh, scale=0.5)
                    # v = tanh(x.w1) * (tanh(x.w2 / 2) + 1) == 2a
                    v = smp.tile([128, BPB // 2], f32, tag="v")
                    nc.vector.scalar_tensor_tensor(
                        v[:], tts[:, :, 1], 1.0, tts[:, :, 0], alu.add, alu.mult
                    )
                    # e = exp(v/2); accumulate denominator partial for chain
                    e_b = smp.tile([128, BPB // 2], bf16, tag="e")
                    nc.scalar.activation(
                        e_b[:], v[:], act.Exp, scale=0.5,
                        accum_out=sAcc[:, h * 5 : h * 5 + 1],
                    )
                    # numerators: sum_n e_n * y_nl
                    for l in range(L):
                        jnk = smp.tile([128, BPB // 2], bf16, tag=f"jnk{l}")
                        nc.vector.scalar_tensor_tensor(
                            jnk[:], py[:, :, l], 1.0, e_b[:], alu.mult, alu.mult,
                            accum_out=sAcc[:, h * 5 + 1 + l : h * 5 + 2 + l],
                        )

            # fold partitions: [1, 40] = ones.T @ sAcc, then ship out
            # (reuses chain 0's PSUM bank -- all 8 banks hold chain tiles)
            psOut = ypool.tile([1, 5 * NCH], f32, tag="ch0")
            nc.tensor.matmul(psOut[:], ones[:], sAcc[:], start=True, stop=True)
            outSb = constp.tile([1, 5 * NCH], f32, tag="outSb")
            nc.scalar.copy(outSb[:], psOut[:])
            nc.sync.dma_start(outp, outSb[:])

    nc.compile()
    return nc


def _get_nc():
    if "nc" not in _NC_CACHE:
        _NC_CACHE["nc"] = _build_nc()
    return _NC_CACHE["nc"]


def _make_in_maps(xs, valid, w1, w2, wf):
    import ml_dtypes

    validf = valid.astype(np.float32)
    xsz = xs.astype(np.float32) * validf[..., None]
    # [D, 6] = [wf.T | 2*w1 | w2], packed per 128-chunk: (p, c*6+j) = W6[c*128+p, j]
    W6 = np.concatenate(
        [wf.astype(np.float32).T, 2.0 * w1.astype(np.float32), w2.astype(np.float32)],
        axis=1,
    )
    wsb = np.ascontiguousarray(
        W6.reshape(C8, 128, 6).transpose(1, 0, 2).reshape(128, C8 * 6)
    ).astype(ml_dtypes.bfloat16)

    in_maps = []
    for c in range(NCORES):
        sh = xsz[:, c * NSH : (c + 1) * NSH, :].reshape(J, D)
        # [blk, nn, chunk, dd] -> [tile, dd, (blk-in-tile, chunk, nn)]
        a = sh.reshape(NBLK, 128, C8, 128).transpose(0, 2, 3, 1)
        a = (
            a.reshape(TD, BPT, C8, 128, 128)
            .transpose(0, 3, 1, 2, 4)
            .reshape(TD, 128, BPT * C8 * 128)
        )
        packed = np.ascontiguousarray(a).astype(ml_dtypes.float8_e3m4)
        in_maps.append({"xsp": packed, "wsb": wsb})
    return in_maps


def _run(xs, valid, w1, w2, wf, trace=False, **kwargs):
    from concourse import bass_utils

    nc = _get_nc()
    in_maps = _make_in_maps(xs, valid, w1, w2, wf)
    res = bass_utils.run_bass_kernel_spmd(
        nc, in_maps, core_ids=list(range(NCORES)), trace=trace, **kwargs
    )
    return res


def _combine(res, valid):
    """Sum per-core partial stats (flash-style unshard) and finalize t/s.

    Invalid instances were zeroed on the device input, so each contributes
    exp(0) = 1 to the denominator partials; subtract their count here.
    """
    tot = np.zeros(5 * NCH, np.float64)
    for c in range(NCORES):
        tot += np.asarray(res.results[c]["out"]).reshape(5 * NCH).astype(np.float64)
    ch = tot.reshape(B, NCH // B, 5).sum(axis=1)  # [bag, (s, t0..t3)]
    n_invalid = (~valid.astype(bool)).sum(axis=1).astype(np.float64)  # [b]
    s = ch[:, 0] - n_invalid                      # [b]
    t = ch[:, 1:]                                 # [b, l]
    return (t / s[:, None]).astype(np.float32)


def kernel(xs, valid, w1, w2, wf):
    xs, valid, w1, w2, wf = (np.asarray(a) for a in (xs, valid, w1, w2, wf))
    res = _run(xs, valid, w1, w2, wf, trace=False)
    return _combine(res, valid)
